# revision 4
# baseline (speedup 1.0000x reference)
"""Trainium2 Bass kernel for masked multi-head attention + out-proj + LayerNorm.

Reference computation (B=4, L=2048, D=1024, H=16, dk=dv=64):
    qh,kh,vh = proj(q),proj(k),proj(v);  s = qh@kh.T/sqrt(dk)
    s = where(edge==0 | mask, -1e15, s); a = softmax(s)
    out = (a@vh)@wo.T + bo + q;  y = layernorm(out)*g + b
    returns (y [B,L,D], attn [H*B,L,L])

Sharding: (batch, L/2 query block) -> 8 cores, zero collectives.
Each core computes its q-rows for all 16 heads end-to-end.
"""

import sys
import numpy as np

sys.path.insert(0, "/opt/trn_rl_repo")

B, L, D = 4, 2048, 1024
H, DK, DV = 16, 64, 64
QL = L // 2          # q rows per core
NQT = QL // 128      # 8 q tiles per core
NKT = L // 128       # 16 key tiles
NEG = -1e15
EPS = 1e-5
SHIFT = -12.0        # exp(s/8 - 12): keeps fp16 P in range

_CACHE = {}


def _build():
    import concourse.bacc as bacc
    import concourse.mybir as mybir
    from concourse.tile import TileContext
    from concourse.masks import make_identity

    f32 = mybir.dt.float32
    f32r = mybir.dt.float32r
    f16 = mybir.dt.float16
    bf16 = mybir.dt.bfloat16
    u8 = mybir.dt.uint8
    i32 = mybir.dt.int32
    AF = mybir.ActivationFunctionType
    Alu = mybir.AluOpType
    import concourse.bass as bass

    nc = bacc.Bacc("TRN2", target_bir_lowering=False, num_devices=8)

    # ---- per-core external tensors ----
    qT_e = nc.declare_dram_parameter("qT", [D, QL], f32, isOutput=False)
    qres_e = nc.declare_dram_parameter("qres", [QL, D], f32, isOutput=False)
    kT_e = nc.declare_dram_parameter("kT", [D, L], f32, isOutput=False)
    vT_e = nc.declare_dram_parameter("vT", [D, L], f32, isOutput=False)
    mask_e = nc.declare_dram_parameter("mask", [QL, L], u8, isOutput=False)
    edge_e = nc.declare_dram_parameter("edge", [QL, L], i32, isOutput=False)
    wqT_e = nc.declare_dram_parameter("wqT", [D, H * DK], f32, isOutput=False)
    wkT_e = nc.declare_dram_parameter("wkT", [D, H * DK], f32, isOutput=False)
    wvT_e = nc.declare_dram_parameter("wvT", [D, H * DV], f32, isOutput=False)
    woT_e = nc.declare_dram_parameter("woT", [H * DV, D], f32, isOutput=False)
    bq_e = nc.declare_dram_parameter("bq", [H * DK], f32, isOutput=False)
    bk_e = nc.declare_dram_parameter("bk", [H * DK], f32, isOutput=False)
    bv_e = nc.declare_dram_parameter("bv", [H * DV], f32, isOutput=False)
    bo_e = nc.declare_dram_parameter("bo", [D], f32, isOutput=False)
    lng_e = nc.declare_dram_parameter("ln_g", [D], f32, isOutput=False)
    lnb_e = nc.declare_dram_parameter("ln_b", [D], f32, isOutput=False)
    y_e = nc.declare_dram_parameter("y", [QL, D], f32, isOutput=True)
    attn_e = nc.declare_dram_parameter("attn", [H, QL, L], f32, isOutput=True)

    def bcast_ap(ext, n):
        return bass.AP(tensor=ext, offset=0, ap=[[0, 128], [1, n]])

    with TileContext(nc) as tc:
        cst = tc.alloc_tile_pool(name="cst", bufs=1)
        res = tc.alloc_tile_pool(name="res", bufs=1)
        dram = tc.alloc_tile_pool(name="dram", bufs=1, space="DRAM")

        # ---- constants ----
        id_f16 = cst.tile([128, 128], f16, tag="idf16")
        make_identity(nc, id_f16)
        id_bf = cst.tile([128, 128], bf16, tag="idbf")
        make_identity(nc, id_bf)
        id_f32 = cst.tile([128, 128], f32, tag="idf32")
        make_identity(nc, id_f32)
        bq_sb = cst.tile([128, 8], f32, tag="bq")
        nc.gpsimd.dma_start(out=bq_sb, in_=bq_e.ap().rearrange("(m p) -> p m", p=128))
        bk_sb = cst.tile([128, 8], f32, tag="bk")
        nc.gpsimd.dma_start(out=bk_sb, in_=bk_e.ap().rearrange("(m p) -> p m", p=128))
        bv_bc = cst.tile([128, D], f32, tag="bv")
        nc.gpsimd.dma_start(out=bv_bc, in_=bcast_ap(bv_e, D))
        lng_bc = cst.tile([128, D], f32, tag="lng")
        nc.gpsimd.dma_start(out=lng_bc, in_=bcast_ap(lng_e, D))
        lnb_bc = cst.tile([128, D], f32, tag="lnb")
        nc.gpsimd.dma_start(out=lnb_bc, in_=bcast_ap(lnb_e, D))
        bo_sb = cst.tile([1, D], f32r, tag="bo")
        nc.sync.dma_start(out=bo_sb, in_=bo_e.ap().bitcast(f32r).rearrange("(o d) -> o d", o=1))
        ones_f = cst.tile([1, 128], f32, tag="onesf")
        nc.vector.memset(ones_f, 1.0)
        ones_sb = cst.tile([1, 128], f32r, tag="ones")
        nc.vector.tensor_copy(ones_sb, ones_f)
        eps_sb = cst.tile([128, 1], f32, tag="eps")
        nc.vector.memset(eps_sb, EPS)
        shift_sb = cst.tile([128, 1], f32, tag="shift")
        nc.vector.memset(shift_sb, SHIFT)

        # ---- persistent projection outputs ----
        khallT = res.tile([128, 8, L], f32r, tag="khallT")   # [d_in_tile, hd_tile, kk]
        vhall = res.tile([128, NKT, H * DV], f16, tag="vhall")  # [kk_in_tile, kk_tile, hd]
        qh_spill = dram.tile([H * DK, QL], f32r, tag="qhsp")
        out_spill = dram.tile([NQT, 128, H * DV], f32, tag="outsp")

        # ================= phase 1: projections =================
        with (
            tc.tile_pool(name="ph1", bufs=1) as ph1,
            tc.tile_pool(name="ph1ev", bufs=3) as ph1ev,
            tc.tile_pool(name="ph1ps", bufs=2, space="PSUM") as ph1ps,
        ):
            # qh = wq @ q^T  -> spill to DRAM
            wqT_sb = ph1.tile([128, 8, H * DK], f32r, tag="wab")
            nc.sync.dma_start(out=wqT_sb, in_=wqT_e.ap().bitcast(f32r).rearrange("(c p) hd -> p c hd", p=128))
            qT_sb = ph1.tile([128, 8, QL], f32r, tag="xab")
            nc.sync.dma_start(out=qT_sb, in_=qT_e.ap().bitcast(f32r).rearrange("(c p) q -> p c q", p=128))
            for m in range(8):
                for n in range(QL // 512):
                    pp = ph1ps.tile([128, 512], f32, tag="pp")
                    for c in range(8):
                        nc.tensor.matmul(pp, wqT_sb[:, c, m * 128:(m + 1) * 128],
                                         qT_sb[:, c, n * 512:(n + 1) * 512],
                                         start=(c == 0), stop=(c == 7))
                    ev = ph1ev.tile([128, 512], f32r, tag="ev")
                    nc.vector.tensor_scalar(ev, pp, bq_sb[:, m:m + 1], None, Alu.add)
                    nc.sync.dma_start(out=qh_spill[m * 128:(m + 1) * 128, n * 512:(n + 1) * 512], in_=ev)
            # kh = wk @ k^T -> resident khallT
            wkT_sb = ph1.tile([128, 8, H * DK], f32r, tag="wab")
            nc.sync.dma_start(out=wkT_sb, in_=wkT_e.ap().bitcast(f32r).rearrange("(c p) hd -> p c hd", p=128))
            for half in range(2):
                kT_h = ph1.tile([128, 8, 1024], f32r, tag="xab")
                nc.sync.dma_start(out=kT_h, in_=kT_e[:, half * 1024:(half + 1) * 1024].bitcast(f32r).rearrange("(c p) kk -> p c kk", p=128))
                for m in range(8):
                    for n in range(2):
                        pp = ph1ps.tile([128, 512], f32, tag="pp")
                        for c in range(8):
                            nc.tensor.matmul(pp, wkT_sb[:, c, m * 128:(m + 1) * 128],
                                             kT_h[:, c, n * 512:(n + 1) * 512],
                                             start=(c == 0), stop=(c == 7))
                        nc.vector.tensor_scalar(khallT[:, m, half * 1024 + n * 512: half * 1024 + (n + 1) * 512],
                                                pp, bk_sb[:, m:m + 1], None, Alu.add)
            # vh = v @ wv^T -> resident vhall [kk, hd] fp16
            wvT_sb = ph1.tile([128, 8, H * DV], f32r, tag="wab")
            nc.sync.dma_start(out=wvT_sb, in_=wvT_e.ap().bitcast(f32r).rearrange("(c p) hd -> p c hd", p=128))
            for half in range(2):
                vT_h = ph1.tile([128, 8, 1024], f32r, tag="xab")
                nc.sync.dma_start(out=vT_h, in_=vT_e[:, half * 1024:(half + 1) * 1024].bitcast(f32r).rearrange("(c p) kk -> p c kk", p=128))
                for t in range(8):          # kk tile within half
                    kt = half * 8 + t
                    for n in range(2):      # hd 512-chunk
                        pp = ph1ps.tile([128, 512], f32, tag="pp")
                        for c in range(8):
                            nc.tensor.matmul(pp, vT_h[:, c, t * 128:(t + 1) * 128],
                                             wvT_sb[:, c, n * 512:(n + 1) * 512],
                                             start=(c == 0), stop=(c == 7))
                        nc.vector.tensor_tensor(out=vhall[:, kt, n * 512:(n + 1) * 512],
                                                in0=pp, in1=bv_bc[:, n * 512:(n + 1) * 512], op=Alu.add)

        # ================= phase 2: attention =================
        with (
            tc.tile_pool(name="ph2a", bufs=2) as ph2a,
            tc.tile_pool(name="ph2b", bufs=2) as ph2b,
            tc.tile_pool(name="ph2c", bufs=3) as ph2c,
            tc.tile_pool(name="sc", bufs=4) as sc,
            tc.tile_pool(name="ps_s", bufs=1, space="PSUM") as ps_s,
            tc.tile_pool(name="ps_t", bufs=1, space="PSUM") as ps_t,
            tc.tile_pool(name="ps_o", bufs=2, space="PSUM") as ps_o,
        ):
            for qt in range(NQT):
                qs = qt * 128
                qh_cur = ph2a.tile([128, 8, 128], f32r, tag="qhc")
                nc.sync.dma_start(out=qh_cur, in_=qh_spill[:, qs:qs + 128].rearrange("(m p) q -> p m q", p=128))
                edge_t = ph2a.tile([128, L], i32, tag="edge")
                nc.sync.dma_start(out=edge_t, in_=edge_e[qs:qs + 128, :])
                mask_t = ph2a.tile([128, L], u8, tag="mask")
                nc.sync.dma_start(out=mask_t, in_=mask_e[qs:qs + 128, :])
                bias_bf = ph2a.tile([128, L], bf16, tag="biasbf")
                for hf in range(2):
                    cols = slice(hf * 1024, (hf + 1) * 1024)
                    bias_e1 = ph2b.tile([128, 1024], f32, tag="biase")
                    nc.vector.tensor_scalar(bias_e1, edge_t[:, cols], 0, NEG, Alu.is_equal, Alu.mult)
                    mask_f1 = ph2b.tile([128, 1024], f32, tag="maskf")
                    nc.vector.tensor_scalar(mask_f1, mask_t[:, cols], NEG, None, Alu.mult)
                    nc.vector.tensor_tensor(out=bias_bf[:, cols], in0=bias_e1, in1=mask_f1, op=Alu.add)
                out_sb = ph2a.tile([128, H * DV], f32, tag="outsb")
                for h in range(H):
                    off = (h % 2) * 64
                    m = h // 2
                    s_ps = ps_s.tile([128, L], f32, tag="s")
                    for j in range(4):
                        js = slice(j * 512, (j + 1) * 512)
                        nc.tensor.matmul(s_ps[:, js], qh_cur[off:off + 64, m, :],
                                         khallT[off:off + 64, m, js], start=True, stop=False)
                        nc.tensor.matmul(s_ps[:, js], id_bf, bias_bf[:, js],
                                         start=False, stop=True)
                    p_f16 = ph2c.tile([128, L], f16, tag="p")
                    rowsum = sc.tile([128, 1], f32, tag="rs")
                    nc.scalar.activation(out=p_f16, in_=s_ps, func=AF.Exp,
                                         scale=1.0 / 8.0, bias=shift_sb, accum_out=rowsum)
                    recip = sc.tile([128, 1], f32, tag="rcp")
                    nc.vector.reciprocal(recip, rowsum)
                    # normalized attention -> DRAM (gpsimd, two halves)
                    for hf in range(2):
                        cols = slice(hf * 1024, (hf + 1) * 1024)
                        attn_sb = ph2b.tile([128, 1024], f32, tag="attn")
                        nc.gpsimd.tensor_scalar(attn_sb, p_f16[:, cols], recip, None, Alu.mult)
                        nc.sync.dma_start(out=attn_e[h, qs:qs + 128, cols], in_=attn_sb)
                    # P^T via PE transpose
                    pt_ps = ps_t.tile([128, NKT, 128], f16, tag="pt")
                    for t in range(NKT):
                        nc.tensor.transpose(pt_ps[:, t, :], p_f16[:, t * 128:(t + 1) * 128], id_f16)
                    pt_sb = ph2c.tile([128, NKT, 128], f16, tag="ptsb")
                    nc.vector.tensor_copy(pt_sb, pt_ps)
                    # PV
                    o_ps = ps_o.tile([128, DV], f32, tag="o")
                    for t in range(NKT):
                        nc.tensor.matmul(o_ps, pt_sb[:, t, :], vhall[:, t, h * 64:(h + 1) * 64],
                                         start=(t == 0), stop=(t == NKT - 1))
                    nc.vector.tensor_scalar(out_sb[:, h * 64:(h + 1) * 64], o_ps, recip, None, Alu.mult)
                nc.sync.dma_start(out=out_spill[qt], in_=out_sb)

        # ================= phase 3: out-proj + layernorm =================
        with (
            tc.tile_pool(name="ph3w", bufs=1) as ph3w,
            tc.tile_pool(name="ph3", bufs=2) as ph3,
            tc.tile_pool(name="sc3", bufs=4) as sc3,
            tc.tile_pool(name="ps3", bufs=2, space="PSUM") as ps3,
            tc.tile_pool(name="ps3t", bufs=2, space="PSUM") as ps3t,
        ):
            woT_sb = ph3w.tile([128, 8, D], f32r, tag="woT")
            nc.sync.dma_start(out=woT_sb, in_=woT_e.ap().bitcast(f32r).rearrange("(m p) d -> p m d", p=128))
            for qt in range(NQT):
                qs = qt * 128
                out_l = ph3.tile([128, H * DV], f32, tag="outl")
                nc.sync.dma_start(out=out_l, in_=out_spill[qt])
                outT = ph3.tile([128, 8, 128], f32r, tag="outT")
                for m in range(8):
                    oT_ps = ps3t.tile([128, 128], f32, tag="oT")
                    nc.tensor.transpose(oT_ps, out_l[:, m * 128:(m + 1) * 128], id_f32)
                    nc.vector.tensor_copy(outT[:, m, :], oT_ps)
                qres_t = ph3.tile([128, D], f32, tag="qres")
                nc.sync.dma_start(out=qres_t, in_=qres_e[qs:qs + 128, :])
                x_sb = ph3.tile([128, D], f32, tag="x")
                for n in range(2):
                    ns = slice(n * 512, (n + 1) * 512)
                    y_ps = ps3.tile([128, 512], f32, tag="yps")
                    for m in range(8):
                        nc.tensor.matmul(y_ps, outT[:, m, :], woT_sb[:, m, ns],
                                         start=(m == 0), stop=False)
                    nc.tensor.matmul(y_ps, ones_sb, bo_sb[:, ns], start=False, stop=True)
                    nc.vector.tensor_tensor(out=x_sb[:, ns], in0=y_ps, in1=qres_t[:, ns], op=Alu.add)
                # layernorm over D=1024 (two 512 subgroups)
                x_g = x_sb.rearrange("p (g d) -> p g d", g=2)
                stats = sc3.tile([128, 2, 6], f32, tag="stats")
                for g in range(2):
                    nc.vector.bn_stats(out=stats[:, g, :], in_=x_g[:, g, :])
                mv = sc3.tile([128, 2], f32, tag="mv")
                nc.vector.bn_aggr(out=mv, in_=stats)
                std = sc3.tile([128, 1], f32, tag="std")
                nc.scalar.activation(out=std, in_=mv[:, 1:2], func=AF.Sqrt, bias=eps_sb, scale=1.0)
                rstd = sc3.tile([128, 1], f32, tag="rstd")
                nc.vector.reciprocal(rstd, std)
                xn = ph3.tile([128, D], f32, tag="xn")
                nc.vector.tensor_scalar(xn, x_sb, mv[:, 0:1], rstd, Alu.subtract, Alu.mult)
                y1 = ph3.tile([128, D], f32, tag="y1")
                nc.vector.tensor_tensor(out=y1, in0=xn, in1=lng_bc, op=Alu.mult)
                y2 = ph3.tile([128, D], f32, tag="y2")
                nc.vector.tensor_tensor(out=y2, in0=y1, in1=lnb_bc, op=Alu.add)
                nc.sync.dma_start(out=y_e[qs:qs + 128, :], in_=y2)

        dram.release()
        res.release()
        cst.release()

    nc.compile()
    return nc


def _get_nc():
    if "nc" not in _CACHE:
        _CACHE["nc"] = _build()
    return _CACHE["nc"]


def kernel(q, k, v, mask, edge, wq, bq, wk, bk, wv, bv, wo, bo, ln_g, ln_b):
    from concourse.bass_utils import run_bass_kernel_spmd

    nc = _get_nc()

    q = np.asarray(q, dtype=np.float32)
    k = np.asarray(k, dtype=np.float32)
    v = np.asarray(v, dtype=np.float32)
    mask_u8 = np.asarray(mask).astype(np.uint8)
    edge_i = np.ascontiguousarray(np.asarray(edge, dtype=np.int32))
    shared = {
        "wqT": np.ascontiguousarray(np.asarray(wq, np.float32).T),
        "wkT": np.ascontiguousarray(np.asarray(wk, np.float32).T),
        "wvT": np.ascontiguousarray(np.asarray(wv, np.float32).T),
        "woT": np.ascontiguousarray(np.asarray(wo, np.float32).T),
        "bq": np.asarray(bq, np.float32), "bk": np.asarray(bk, np.float32),
        "bv": np.asarray(bv, np.float32), "bo": np.asarray(bo, np.float32),
        "ln_g": np.asarray(ln_g, np.float32), "ln_b": np.asarray(ln_b, np.float32),
    }
    in_maps = []
    for core in range(8):
        b, q0 = core // 2, (core % 2) * QL
        m = dict(shared)
        m["qT"] = np.ascontiguousarray(q[b, q0:q0 + QL, :].T)
        m["qres"] = np.ascontiguousarray(q[b, q0:q0 + QL, :])
        m["kT"] = np.ascontiguousarray(k[b].T)
        m["vT"] = np.ascontiguousarray(v[b].T)
        m["mask"] = np.ascontiguousarray(mask_u8[b, q0:q0 + QL, :])
        m["edge"] = np.ascontiguousarray(edge_i[b, q0:q0 + QL, :])
        in_maps.append(m)

    results = run_bass_kernel_spmd(nc, in_maps, list(range(8))).results

    y = np.empty((B, L, D), np.float32)
    attn = np.empty((H * B, L, L), np.float32)
    for core in range(8):
        b, q0 = core // 2, (core % 2) * QL
        y[b, q0:q0 + QL, :] = results[core]["y"]
        a = results[core]["attn"]          # [H, QL, L]
        for h in range(H):
            attn[h * B + b, q0:q0 + QL, :] = a[h]
    return y, attn


# revision 6
# speedup vs baseline: 2.4959x; 2.4959x over previous
"""Trainium2 Bass kernel for masked multi-head attention + out-proj + LayerNorm.

Reference computation (B=4, L=2048, D=1024, H=16, dk=dv=64):
    qh,kh,vh = proj(q),proj(k),proj(v);  s = qh@kh.T/sqrt(dk)
    s = where(edge==0 | mask, -1e15, s); a = softmax(s)
    out = (a@vh)@wo.T + bo + q;  y = layernorm(out)*g + b
    returns (y [B,L,D], attn [H*B,L,L])

Sharding: (batch, L/2 query block) -> 8 cores, zero collectives.
Each core computes its q-rows for all 16 heads end-to-end.
"""

import sys
import numpy as np

sys.path.insert(0, "/opt/trn_rl_repo")

B, L, D = 4, 2048, 1024
H, DK, DV = 16, 64, 64
QL = L // 2          # q rows per core
NQT = QL // 128      # 8 q tiles per core
NKT = L // 128       # 16 key tiles
NEG = -1e15
EPS = 1e-5
SHIFT = -12.0        # exp(s/8 - 12): keeps fp16 P in range

_CACHE = {}


def _build():
    import concourse.bacc as bacc
    import concourse.mybir as mybir
    from concourse.tile import TileContext
    from concourse.masks import make_identity

    f32 = mybir.dt.float32
    f32r = mybir.dt.float32r
    f16 = mybir.dt.float16
    bf16 = mybir.dt.bfloat16
    u8 = mybir.dt.uint8
    i32 = mybir.dt.int32
    AF = mybir.ActivationFunctionType
    Alu = mybir.AluOpType
    import concourse.bass as bass

    nc = bacc.Bacc("TRN2", target_bir_lowering=False, num_devices=8)

    # ---- per-core external tensors ----
    qT_e = nc.declare_dram_parameter("qT", [D, QL], f32, isOutput=False)
    qres_e = nc.declare_dram_parameter("qres", [QL, D], f32, isOutput=False)
    kT_e = nc.declare_dram_parameter("kT", [D, L], f32, isOutput=False)
    vT_e = nc.declare_dram_parameter("vT", [D, L], f32, isOutput=False)
    mask_e = nc.declare_dram_parameter("mask", [QL, L], u8, isOutput=False)
    edge_e = nc.declare_dram_parameter("edge", [QL, L], i32, isOutput=False)
    wqT_e = nc.declare_dram_parameter("wqT", [D, H * DK], f32, isOutput=False)
    wkT_e = nc.declare_dram_parameter("wkT", [D, H * DK], f32, isOutput=False)
    wvT_e = nc.declare_dram_parameter("wvT", [D, H * DV], f32, isOutput=False)
    woT_e = nc.declare_dram_parameter("woT", [H * DV, D], f32, isOutput=False)
    bq_e = nc.declare_dram_parameter("bq", [H * DK], f32, isOutput=False)
    bk_e = nc.declare_dram_parameter("bk", [H * DK], f32, isOutput=False)
    bv_e = nc.declare_dram_parameter("bv", [H * DV], f32, isOutput=False)
    bo_e = nc.declare_dram_parameter("bo", [D], f32, isOutput=False)
    lng_e = nc.declare_dram_parameter("ln_g", [D], f32, isOutput=False)
    lnb_e = nc.declare_dram_parameter("ln_b", [D], f32, isOutput=False)
    y_e = nc.declare_dram_parameter("y", [QL, D], f32, isOutput=True)
    attn_e = nc.declare_dram_parameter("attn", [H, QL, L], f32, isOutput=True)

    def bcast_ap(ext, n):
        return bass.AP(tensor=ext, offset=0, ap=[[0, 128], [1, n]])

    with TileContext(nc) as tc:
        cst = tc.alloc_tile_pool(name="cst", bufs=1)
        res = tc.alloc_tile_pool(name="res", bufs=1)
        dram = tc.alloc_tile_pool(name="dram", bufs=1, space="DRAM")

        # ---- constants ----
        id_f16 = cst.tile([128, 128], f16, tag="idf16")
        make_identity(nc, id_f16)
        id_bf = cst.tile([128, 128], bf16, tag="idbf")
        make_identity(nc, id_bf)
        id_f32 = cst.tile([128, 128], f32, tag="idf32")
        make_identity(nc, id_f32)
        bq_sb = cst.tile([128, 8], f32, tag="bq")
        nc.gpsimd.dma_start(out=bq_sb, in_=bq_e.ap().rearrange("(m p) -> p m", p=128))
        bk_sb = cst.tile([128, 8], f32, tag="bk")
        nc.gpsimd.dma_start(out=bk_sb, in_=bk_e.ap().rearrange("(m p) -> p m", p=128))
        bv_bc = cst.tile([128, D], f32, tag="bv")
        nc.gpsimd.dma_start(out=bv_bc, in_=bcast_ap(bv_e, D))
        lng_bc = cst.tile([128, D], f32, tag="lng")
        nc.gpsimd.dma_start(out=lng_bc, in_=bcast_ap(lng_e, D))
        lnb_bc = cst.tile([128, D], f32, tag="lnb")
        nc.gpsimd.dma_start(out=lnb_bc, in_=bcast_ap(lnb_e, D))
        bo_sb = cst.tile([1, D], f32r, tag="bo")
        nc.sync.dma_start(out=bo_sb, in_=bo_e.ap().bitcast(f32r).rearrange("(o d) -> o d", o=1))
        ones_f = cst.tile([1, 128], f32, tag="onesf")
        nc.vector.memset(ones_f, 1.0)
        ones_sb = cst.tile([1, 128], f32r, tag="ones")
        nc.vector.tensor_copy(ones_sb, ones_f)
        eps_sb = cst.tile([128, 1], f32, tag="eps")
        nc.vector.memset(eps_sb, EPS)
        shift_sb = cst.tile([128, 1], f32, tag="shift")
        nc.vector.memset(shift_sb, SHIFT)

        # ---- persistent projection outputs ----
        khallT = res.tile([128, 8, L], f32r, tag="khallT")   # [d_in_tile, hd_tile, kk]
        vhall = res.tile([128, NKT, H * DV], f16, tag="vhall")  # [kk_in_tile, kk_tile, hd]
        qh_spill = dram.tile([H * DK, QL], f32r, tag="qhsp")
        out_spill = dram.tile([NQT, 128, H * DV], f32, tag="outsp")

        # ================= phase 1: projections =================
        with (
            tc.tile_pool(name="ph1", bufs=1) as ph1,
            tc.tile_pool(name="ph1ev", bufs=3) as ph1ev,
            tc.tile_pool(name="ph1ps", bufs=2, space="PSUM") as ph1ps,
        ):
            # qh = wq @ q^T  -> spill to DRAM
            wqT_sb = ph1.tile([128, 8, H * DK], f32r, tag="wab")
            nc.sync.dma_start(out=wqT_sb, in_=wqT_e.ap().bitcast(f32r).rearrange("(c p) hd -> p c hd", p=128))
            qT_sb = ph1.tile([128, 8, QL], f32r, tag="xab")
            nc.sync.dma_start(out=qT_sb, in_=qT_e.ap().bitcast(f32r).rearrange("(c p) q -> p c q", p=128))
            for m in range(8):
                for n in range(QL // 512):
                    pp = ph1ps.tile([128, 512], f32, tag="pp")
                    for c in range(8):
                        nc.tensor.matmul(pp, wqT_sb[:, c, m * 128:(m + 1) * 128],
                                         qT_sb[:, c, n * 512:(n + 1) * 512],
                                         start=(c == 0), stop=(c == 7))
                    ev = ph1ev.tile([128, 512], f32r, tag="ev")
                    nc.vector.tensor_scalar(ev, pp, bq_sb[:, m:m + 1], None, Alu.add)
                    nc.sync.dma_start(out=qh_spill[m * 128:(m + 1) * 128, n * 512:(n + 1) * 512], in_=ev)
            # kh = wk @ k^T -> resident khallT
            wkT_sb = ph1.tile([128, 8, H * DK], f32r, tag="wab")
            nc.sync.dma_start(out=wkT_sb, in_=wkT_e.ap().bitcast(f32r).rearrange("(c p) hd -> p c hd", p=128))
            for half in range(2):
                kT_h = ph1.tile([128, 8, 1024], f32r, tag="xab")
                nc.sync.dma_start(out=kT_h, in_=kT_e[:, half * 1024:(half + 1) * 1024].bitcast(f32r).rearrange("(c p) kk -> p c kk", p=128))
                for m in range(8):
                    for n in range(2):
                        pp = ph1ps.tile([128, 512], f32, tag="pp")
                        for c in range(8):
                            nc.tensor.matmul(pp, wkT_sb[:, c, m * 128:(m + 1) * 128],
                                             kT_h[:, c, n * 512:(n + 1) * 512],
                                             start=(c == 0), stop=(c == 7))
                        nc.vector.tensor_scalar(khallT[:, m, half * 1024 + n * 512: half * 1024 + (n + 1) * 512],
                                                pp, bk_sb[:, m:m + 1], None, Alu.add)
            # vh = v @ wv^T -> resident vhall [kk, hd] fp16
            wvT_sb = ph1.tile([128, 8, H * DV], f32r, tag="wab")
            nc.sync.dma_start(out=wvT_sb, in_=wvT_e.ap().bitcast(f32r).rearrange("(c p) hd -> p c hd", p=128))
            for half in range(2):
                vT_h = ph1.tile([128, 8, 1024], f32r, tag="xab")
                nc.sync.dma_start(out=vT_h, in_=vT_e[:, half * 1024:(half + 1) * 1024].bitcast(f32r).rearrange("(c p) kk -> p c kk", p=128))
                for t in range(8):          # kk tile within half
                    kt = half * 8 + t
                    for n in range(2):      # hd 512-chunk
                        pp = ph1ps.tile([128, 512], f32, tag="pp")
                        for c in range(8):
                            nc.tensor.matmul(pp, vT_h[:, c, t * 128:(t + 1) * 128],
                                             wvT_sb[:, c, n * 512:(n + 1) * 512],
                                             start=(c == 0), stop=(c == 7))
                        nc.vector.tensor_tensor(out=vhall[:, kt, n * 512:(n + 1) * 512],
                                                in0=pp, in1=bv_bc[:, n * 512:(n + 1) * 512], op=Alu.add)

        # ================= phase 2: attention =================
        with (
            tc.tile_pool(name="ph2a", bufs=2) as ph2a,
            tc.tile_pool(name="ph2b", bufs=2) as ph2b,
            tc.tile_pool(name="ph2c", bufs=2) as ph2c,
            tc.tile_pool(name="sc", bufs=4) as sc,
            tc.tile_pool(name="ps_s", bufs=1, space="PSUM") as ps_s,
            tc.tile_pool(name="ps_t", bufs=1, space="PSUM") as ps_t,
            tc.tile_pool(name="ps_o", bufs=2, space="PSUM") as ps_o,
        ):
            for qt in range(NQT):
                qs = qt * 128
                qh_cur = ph2a.tile([128, 8, 128], f32r, tag="qhc")
                nc.sync.dma_start(out=qh_cur, in_=qh_spill[:, qs:qs + 128].rearrange("(m p) q -> p m q", p=128))
                edge_t = ph2a.tile([128, L], f32, tag="edge")
                nc.gpsimd.dma_start(out=edge_t, in_=edge_e[qs:qs + 128, :])
                mask_t = ph2a.tile([128, L], u8, tag="mask")
                nc.sync.dma_start(out=mask_t, in_=mask_e[qs:qs + 128, :])
                bias_bf = ph2a.tile([128, L], bf16, tag="biasbf")
                for hf in range(2):
                    cols = slice(hf * 1024, (hf + 1) * 1024)
                    bias_e1 = ph2b.tile([128, 1024], f32, tag="biase")
                    nc.vector.tensor_scalar(bias_e1, edge_t[:, cols], 0.0, NEG, Alu.is_equal, Alu.mult)
                    mask_f1 = ph2b.tile([128, 1024], f32, tag="maskf")
                    nc.vector.tensor_scalar(mask_f1, mask_t[:, cols], NEG, None, Alu.mult)
                    nc.vector.tensor_tensor(out=bias_bf[:, cols], in0=bias_e1, in1=mask_f1, op=Alu.add)
                out_sb = ph2a.tile([128, H * DV], f32, tag="outsb")
                for h in range(H):
                    off = (h % 2) * 64
                    m = h // 2
                    s_ps = ps_s.tile([128, L], f32, tag="s")
                    for j in range(4):
                        js = slice(j * 512, (j + 1) * 512)
                        nc.tensor.matmul(s_ps[:, js], qh_cur[off:off + 64, m, :],
                                         khallT[off:off + 64, m, js], start=True, stop=False)
                        nc.tensor.matmul(s_ps[:, js], id_bf, bias_bf[:, js],
                                         start=False, stop=True)
                    p_f16 = ph2c.tile([128, L], f16, tag="p")
                    rowsum = sc.tile([128, 1], f32, tag="rs")
                    nc.scalar.activation(out=p_f16, in_=s_ps, func=AF.Exp,
                                         scale=1.0 / 8.0, bias=shift_sb, accum_out=rowsum)
                    recip = sc.tile([128, 1], f32, tag="rcp")
                    nc.vector.reciprocal(recip, rowsum)
                    # normalized attention: DVE 4x fp16, cast to f32 during DMA store
                    attn_sb = ph2c.tile([128, L], f16, tag="attn")
                    nc.vector.tensor_scalar(attn_sb, p_f16, recip, None, Alu.mult)
                    nc.gpsimd.dma_start(out=attn_e[h, qs:qs + 128, :], in_=attn_sb)
                    # P^T via PE transpose
                    pt_ps = ps_t.tile([128, NKT, 128], f16, tag="pt")
                    for t in range(NKT):
                        nc.tensor.transpose(pt_ps[:, t, :], p_f16[:, t * 128:(t + 1) * 128], id_f16)
                    pt_sb = ph2c.tile([128, NKT, 128], f16, tag="ptsb")
                    nc.vector.tensor_copy(pt_sb, pt_ps)
                    # PV
                    o_ps = ps_o.tile([128, DV], f32, tag="o")
                    for t in range(NKT):
                        nc.tensor.matmul(o_ps, pt_sb[:, t, :], vhall[:, t, h * 64:(h + 1) * 64],
                                         start=(t == 0), stop=(t == NKT - 1))
                    nc.vector.tensor_scalar(out_sb[:, h * 64:(h + 1) * 64], o_ps, recip, None, Alu.mult)
                nc.sync.dma_start(out=out_spill[qt], in_=out_sb)

        # ================= phase 3: out-proj + layernorm =================
        with (
            tc.tile_pool(name="ph3w", bufs=1) as ph3w,
            tc.tile_pool(name="ph3", bufs=2) as ph3,
            tc.tile_pool(name="sc3", bufs=4) as sc3,
            tc.tile_pool(name="ps3", bufs=2, space="PSUM") as ps3,
            tc.tile_pool(name="ps3t", bufs=2, space="PSUM") as ps3t,
        ):
            woT_sb = ph3w.tile([128, 8, D], f32r, tag="woT")
            nc.sync.dma_start(out=woT_sb, in_=woT_e.ap().bitcast(f32r).rearrange("(m p) d -> p m d", p=128))
            for qt in range(NQT):
                qs = qt * 128
                out_l = ph3.tile([128, H * DV], f32, tag="outl")
                nc.sync.dma_start(out=out_l, in_=out_spill[qt])
                outT = ph3.tile([128, 8, 128], f32r, tag="outT")
                for m in range(8):
                    oT_ps = ps3t.tile([128, 128], f32, tag="oT")
                    nc.tensor.transpose(oT_ps, out_l[:, m * 128:(m + 1) * 128], id_f32)
                    nc.vector.tensor_copy(outT[:, m, :], oT_ps)
                qres_t = ph3.tile([128, D], f32, tag="qres")
                nc.sync.dma_start(out=qres_t, in_=qres_e[qs:qs + 128, :])
                x_sb = ph3.tile([128, D], f32, tag="x")
                for n in range(2):
                    ns = slice(n * 512, (n + 1) * 512)
                    y_ps = ps3.tile([128, 512], f32, tag="yps")
                    for m in range(8):
                        nc.tensor.matmul(y_ps, outT[:, m, :], woT_sb[:, m, ns],
                                         start=(m == 0), stop=False)
                    nc.tensor.matmul(y_ps, ones_sb, bo_sb[:, ns], start=False, stop=True)
                    nc.vector.tensor_tensor(out=x_sb[:, ns], in0=y_ps, in1=qres_t[:, ns], op=Alu.add)
                # layernorm over D=1024 (two 512 subgroups)
                x_g = x_sb.rearrange("p (g d) -> p g d", g=2)
                stats = sc3.tile([128, 2, 6], f32, tag="stats")
                for g in range(2):
                    nc.vector.bn_stats(out=stats[:, g, :], in_=x_g[:, g, :])
                mv = sc3.tile([128, 2], f32, tag="mv")
                nc.vector.bn_aggr(out=mv, in_=stats)
                std = sc3.tile([128, 1], f32, tag="std")
                nc.scalar.activation(out=std, in_=mv[:, 1:2], func=AF.Sqrt, bias=eps_sb, scale=1.0)
                rstd = sc3.tile([128, 1], f32, tag="rstd")
                nc.vector.reciprocal(rstd, std)
                xn = ph3.tile([128, D], f32, tag="xn")
                nc.vector.tensor_scalar(xn, x_sb, mv[:, 0:1], rstd, Alu.subtract, Alu.mult)
                y1 = ph3.tile([128, D], f32, tag="y1")
                nc.vector.tensor_tensor(out=y1, in0=xn, in1=lng_bc, op=Alu.mult)
                y2 = ph3.tile([128, D], f32, tag="y2")
                nc.vector.tensor_tensor(out=y2, in0=y1, in1=lnb_bc, op=Alu.add)
                nc.sync.dma_start(out=y_e[qs:qs + 128, :], in_=y2)

        dram.release()
        res.release()
        cst.release()

    nc.compile()
    return nc


def _get_nc():
    if "nc" not in _CACHE:
        _CACHE["nc"] = _build()
    return _CACHE["nc"]


def kernel(q, k, v, mask, edge, wq, bq, wk, bk, wv, bv, wo, bo, ln_g, ln_b):
    from concourse.bass_utils import run_bass_kernel_spmd

    nc = _get_nc()

    q = np.asarray(q, dtype=np.float32)
    k = np.asarray(k, dtype=np.float32)
    v = np.asarray(v, dtype=np.float32)
    mask_u8 = np.asarray(mask).astype(np.uint8)
    edge_i = np.ascontiguousarray(np.asarray(edge, dtype=np.int32))
    shared = {
        "wqT": np.ascontiguousarray(np.asarray(wq, np.float32).T),
        "wkT": np.ascontiguousarray(np.asarray(wk, np.float32).T),
        "wvT": np.ascontiguousarray(np.asarray(wv, np.float32).T),
        "woT": np.ascontiguousarray(np.asarray(wo, np.float32).T),
        "bq": np.asarray(bq, np.float32), "bk": np.asarray(bk, np.float32),
        "bv": np.asarray(bv, np.float32), "bo": np.asarray(bo, np.float32),
        "ln_g": np.asarray(ln_g, np.float32), "ln_b": np.asarray(ln_b, np.float32),
    }
    in_maps = []
    for core in range(8):
        b, q0 = core // 2, (core % 2) * QL
        m = dict(shared)
        m["qT"] = np.ascontiguousarray(q[b, q0:q0 + QL, :].T)
        m["qres"] = np.ascontiguousarray(q[b, q0:q0 + QL, :])
        m["kT"] = np.ascontiguousarray(k[b].T)
        m["vT"] = np.ascontiguousarray(v[b].T)
        m["mask"] = np.ascontiguousarray(mask_u8[b, q0:q0 + QL, :])
        m["edge"] = np.ascontiguousarray(edge_i[b, q0:q0 + QL, :])
        in_maps.append(m)

    results = run_bass_kernel_spmd(nc, in_maps, list(range(8))).results

    y = np.empty((B, L, D), np.float32)
    attn = np.empty((H * B, L, L), np.float32)
    for core in range(8):
        b, q0 = core // 2, (core % 2) * QL
        y[b, q0:q0 + QL, :] = results[core]["y"]
        a = results[core]["attn"]          # [H, QL, L]
        for h in range(H):
            attn[h * B + b, q0:q0 + QL, :] = a[h]
    return y, attn


# revision 7
# speedup vs baseline: 2.7346x; 1.0956x over previous
"""Trainium2 Bass kernel for masked multi-head attention + out-proj + LayerNorm.

Reference computation (B=4, L=2048, D=1024, H=16, dk=dv=64):
    qh,kh,vh = proj(q),proj(k),proj(v);  s = qh@kh.T/sqrt(dk)
    s = where(edge==0 | mask, -1e15, s); a = softmax(s)
    out = (a@vh)@wo.T + bo + q;  y = layernorm(out)*g + b
    returns (y [B,L,D], attn [H*B,L,L])

Sharding: (batch, L/2 query block) -> 8 cores, zero collectives.
Each core computes its q-rows for all 16 heads end-to-end.
"""

import sys
import numpy as np

sys.path.insert(0, "/opt/trn_rl_repo")

B, L, D = 4, 2048, 1024
H, DK, DV = 16, 64, 64
QL = L // 2          # q rows per core
NQT = QL // 128      # 8 q tiles per core
NKT = L // 128       # 16 key tiles
NEG = -1e15
EPS = 1e-5
SHIFT = -12.0        # exp(s/8 - 12): keeps fp16 P in range

_CACHE = {}


def _build():
    import concourse.bacc as bacc
    import concourse.mybir as mybir
    from concourse.tile import TileContext
    from concourse.masks import make_identity

    f32 = mybir.dt.float32
    f32r = mybir.dt.float32r
    f16 = mybir.dt.float16
    bf16 = mybir.dt.bfloat16
    u8 = mybir.dt.uint8
    i32 = mybir.dt.int32
    AF = mybir.ActivationFunctionType
    Alu = mybir.AluOpType
    import concourse.bass as bass

    nc = bacc.Bacc("TRN2", target_bir_lowering=False, num_devices=8)

    # ---- per-core external tensors ----
    qT_e = nc.declare_dram_parameter("qT", [D, QL], f32, isOutput=False)
    qres_e = nc.declare_dram_parameter("qres", [QL, D], f32, isOutput=False)
    kT_e = nc.declare_dram_parameter("kT", [D, L], f32, isOutput=False)
    vT_e = nc.declare_dram_parameter("vT", [D, L], f32, isOutput=False)
    mask_e = nc.declare_dram_parameter("mask", [QL, L], u8, isOutput=False)
    edge_e = nc.declare_dram_parameter("edge", [QL, L], i32, isOutput=False)
    wqT_e = nc.declare_dram_parameter("wqT", [D, H * DK], f32, isOutput=False)
    wkT_e = nc.declare_dram_parameter("wkT", [D, H * DK], f32, isOutput=False)
    wvT_e = nc.declare_dram_parameter("wvT", [D, H * DV], f32, isOutput=False)
    woT_e = nc.declare_dram_parameter("woT", [H * DV, D], f32, isOutput=False)
    bq_e = nc.declare_dram_parameter("bq", [H * DK], f32, isOutput=False)
    bk_e = nc.declare_dram_parameter("bk", [H * DK], f32, isOutput=False)
    bv_e = nc.declare_dram_parameter("bv", [H * DV], f32, isOutput=False)
    bo_e = nc.declare_dram_parameter("bo", [D], f32, isOutput=False)
    lng_e = nc.declare_dram_parameter("ln_g", [D], f32, isOutput=False)
    lnb_e = nc.declare_dram_parameter("ln_b", [D], f32, isOutput=False)
    y_e = nc.declare_dram_parameter("y", [QL, D], f32, isOutput=True)
    attn_e = nc.declare_dram_parameter("attn", [H, QL, L], f32, isOutput=True)

    def bcast_ap(ext, n):
        return bass.AP(tensor=ext, offset=0, ap=[[0, 128], [1, n]])

    with TileContext(nc) as tc:
        cst = tc.alloc_tile_pool(name="cst", bufs=1)
        res = tc.alloc_tile_pool(name="res", bufs=1)
        dram = tc.alloc_tile_pool(name="dram", bufs=1, space="DRAM")

        # ---- constants ----
        id_f16 = cst.tile([128, 128], f16, tag="idf16")
        make_identity(nc, id_f16)
        id_bf = cst.tile([128, 128], bf16, tag="idbf")
        make_identity(nc, id_bf)
        id_f32 = cst.tile([128, 128], f32, tag="idf32")
        make_identity(nc, id_f32)
        bq_sb = cst.tile([128, 8], f32, tag="bq")
        nc.gpsimd.dma_start(out=bq_sb, in_=bq_e.ap().rearrange("(m p) -> p m", p=128))
        bk_sb = cst.tile([128, 8], f32, tag="bk")
        nc.gpsimd.dma_start(out=bk_sb, in_=bk_e.ap().rearrange("(m p) -> p m", p=128))
        bv_bc = cst.tile([128, D], f32, tag="bv")
        nc.gpsimd.dma_start(out=bv_bc, in_=bcast_ap(bv_e, D))
        lng_bc = cst.tile([128, D], f32, tag="lng")
        nc.gpsimd.dma_start(out=lng_bc, in_=bcast_ap(lng_e, D))
        lnb_bc = cst.tile([128, D], f32, tag="lnb")
        nc.gpsimd.dma_start(out=lnb_bc, in_=bcast_ap(lnb_e, D))
        bo_sb = cst.tile([1, D], f32r, tag="bo")
        nc.sync.dma_start(out=bo_sb, in_=bo_e.ap().bitcast(f32r).rearrange("(o d) -> o d", o=1))
        ones_f = cst.tile([1, 128], f32, tag="onesf")
        nc.vector.memset(ones_f, 1.0)
        ones_sb = cst.tile([1, 128], f32r, tag="ones")
        nc.vector.tensor_copy(ones_sb, ones_f)
        eps_sb = cst.tile([128, 1], f32, tag="eps")
        nc.vector.memset(eps_sb, EPS)
        shift_sb = cst.tile([128, 1], f32, tag="shift")
        nc.vector.memset(shift_sb, SHIFT)

        # ---- persistent projection outputs ----
        khallT = res.tile([128, 8, L], f16, tag="khallT")   # [d_in_tile, hd_tile, kk]
        vhall = res.tile([128, NKT, H * DV], f16, tag="vhall")  # [kk_in_tile, kk_tile, hd]
        qh_spill = dram.tile([H * DK, QL], f16, tag="qhsp")
        out_spill = dram.tile([NQT, 128, H * DV], f32, tag="outsp")

        # ================= phase 1: projections =================
        with (
            tc.tile_pool(name="ph1", bufs=1) as ph1,
            tc.tile_pool(name="ph1ev", bufs=3) as ph1ev,
            tc.tile_pool(name="ph1ps", bufs=2, space="PSUM") as ph1ps,
        ):
            # qh = wq @ q^T  -> spill to DRAM
            wqT_sb = ph1.tile([128, 8, H * DK], f32r, tag="wab")
            nc.sync.dma_start(out=wqT_sb, in_=wqT_e.ap().bitcast(f32r).rearrange("(c p) hd -> p c hd", p=128))
            qT_sb = ph1.tile([128, 8, QL], f32r, tag="xab")
            nc.sync.dma_start(out=qT_sb, in_=qT_e.ap().bitcast(f32r).rearrange("(c p) q -> p c q", p=128))
            for m in range(8):
                for n in range(QL // 512):
                    pp = ph1ps.tile([128, 512], f32, tag="pp")
                    for c in range(8):
                        nc.tensor.matmul(pp, wqT_sb[:, c, m * 128:(m + 1) * 128],
                                         qT_sb[:, c, n * 512:(n + 1) * 512],
                                         start=(c == 0), stop=(c == 7))
                    ev = ph1ev.tile([128, 512], f16, tag="ev")
                    nc.vector.tensor_scalar(ev, pp, bq_sb[:, m:m + 1], None, Alu.add)
                    nc.sync.dma_start(out=qh_spill[m * 128:(m + 1) * 128, n * 512:(n + 1) * 512], in_=ev)
            # kh = wk @ k^T -> resident khallT
            wkT_sb = ph1.tile([128, 8, H * DK], f32r, tag="wab")
            nc.sync.dma_start(out=wkT_sb, in_=wkT_e.ap().bitcast(f32r).rearrange("(c p) hd -> p c hd", p=128))
            for half in range(2):
                kT_h = ph1.tile([128, 8, 1024], f32r, tag="xab")
                nc.sync.dma_start(out=kT_h, in_=kT_e[:, half * 1024:(half + 1) * 1024].bitcast(f32r).rearrange("(c p) kk -> p c kk", p=128))
                for m in range(8):
                    for n in range(2):
                        pp = ph1ps.tile([128, 512], f32, tag="pp")
                        for c in range(8):
                            nc.tensor.matmul(pp, wkT_sb[:, c, m * 128:(m + 1) * 128],
                                             kT_h[:, c, n * 512:(n + 1) * 512],
                                             start=(c == 0), stop=(c == 7))
                        nc.vector.tensor_scalar(khallT[:, m, half * 1024 + n * 512: half * 1024 + (n + 1) * 512],
                                                pp, bk_sb[:, m:m + 1], None, Alu.add)
            # vh = v @ wv^T -> resident vhall [kk, hd] fp16
            wvT_sb = ph1.tile([128, 8, H * DV], f32r, tag="wab")
            nc.sync.dma_start(out=wvT_sb, in_=wvT_e.ap().bitcast(f32r).rearrange("(c p) hd -> p c hd", p=128))
            for half in range(2):
                vT_h = ph1.tile([128, 8, 1024], f32r, tag="xab")
                nc.sync.dma_start(out=vT_h, in_=vT_e[:, half * 1024:(half + 1) * 1024].bitcast(f32r).rearrange("(c p) kk -> p c kk", p=128))
                for t in range(8):          # kk tile within half
                    kt = half * 8 + t
                    for n in range(2):      # hd 512-chunk
                        pp = ph1ps.tile([128, 512], f32, tag="pp")
                        for c in range(8):
                            nc.tensor.matmul(pp, vT_h[:, c, t * 128:(t + 1) * 128],
                                             wvT_sb[:, c, n * 512:(n + 1) * 512],
                                             start=(c == 0), stop=(c == 7))
                        nc.vector.tensor_tensor(out=vhall[:, kt, n * 512:(n + 1) * 512],
                                                in0=pp, in1=bv_bc[:, n * 512:(n + 1) * 512], op=Alu.add)

        # ================= phase 2: attention =================
        with (
            tc.tile_pool(name="ph2a", bufs=2) as ph2a,
            tc.tile_pool(name="ph2b", bufs=2) as ph2b,
            tc.tile_pool(name="ph2c", bufs=3) as ph2c,
            tc.tile_pool(name="sc", bufs=4) as sc,
            tc.tile_pool(name="ps_s", bufs=1, space="PSUM") as ps_s,
            tc.tile_pool(name="ps_t", bufs=1, space="PSUM") as ps_t,
            tc.tile_pool(name="ps_o", bufs=2, space="PSUM") as ps_o,
        ):
            for qt in range(NQT):
                qs = qt * 128
                qh_cur = ph2a.tile([128, 8, 128], f16, tag="qhc")
                nc.sync.dma_start(out=qh_cur, in_=qh_spill[:, qs:qs + 128].rearrange("(m p) q -> p m q", p=128))
                edge_t = ph2a.tile([128, L], f32, tag="edge")
                nc.gpsimd.dma_start(out=edge_t, in_=edge_e[qs:qs + 128, :])
                mask_t = ph2a.tile([128, L], u8, tag="mask")
                nc.sync.dma_start(out=mask_t, in_=mask_e[qs:qs + 128, :])
                bias_bf = ph2a.tile([128, L], bf16, tag="biasbf")
                for hf in range(2):
                    cols = slice(hf * 1024, (hf + 1) * 1024)
                    bias_e1 = ph2b.tile([128, 1024], f32, tag="biase")
                    nc.vector.tensor_scalar(bias_e1, edge_t[:, cols], 0.0, NEG, Alu.is_equal, Alu.mult)
                    mask_f1 = ph2b.tile([128, 1024], f32, tag="maskf")
                    nc.vector.tensor_scalar(mask_f1, mask_t[:, cols], NEG, None, Alu.mult)
                    nc.vector.tensor_tensor(out=bias_bf[:, cols], in0=bias_e1, in1=mask_f1, op=Alu.add)
                out_sb = ph2a.tile([128, H * DV], f32, tag="outsb")
                for h in range(H):
                    off = (h % 2) * 64
                    m = h // 2
                    s_ps = ps_s.tile([128, L], f32, tag="s")
                    for j in range(4):
                        js = slice(j * 512, (j + 1) * 512)
                        nc.tensor.matmul(s_ps[:, js], qh_cur[off:off + 64, m, :],
                                         khallT[off:off + 64, m, js], start=True, stop=False)
                        nc.tensor.matmul(s_ps[:, js], id_bf, bias_bf[:, js],
                                         start=False, stop=True)
                    p_f16 = ph2c.tile([128, L], f16, tag="p")
                    rowsum = sc.tile([128, 1], f32, tag="rs")
                    nc.scalar.activation(out=p_f16, in_=s_ps, func=AF.Exp,
                                         scale=1.0 / 8.0, bias=shift_sb, accum_out=rowsum)
                    recip = sc.tile([128, 1], f32, tag="rcp")
                    nc.vector.reciprocal(recip, rowsum)
                    # normalized attention: DVE 4x fp16, cast to f32 during DMA store
                    attn_sb = ph2c.tile([128, L], f16, tag="attn")
                    nc.vector.tensor_scalar(attn_sb, p_f16, recip, None, Alu.mult)
                    nc.gpsimd.dma_start(out=attn_e[h, qs:qs + 128, :], in_=attn_sb)
                    # P^T via PE transpose
                    pt_ps = ps_t.tile([128, NKT, 128], f16, tag="pt")
                    for t in range(NKT):
                        nc.tensor.transpose(pt_ps[:, t, :], p_f16[:, t * 128:(t + 1) * 128], id_f16)
                    pt_sb = ph2c.tile([128, NKT, 128], f16, tag="ptsb")
                    nc.vector.tensor_copy(pt_sb[:, 0:8, :], pt_ps[:, 0:8, :])
                    nc.scalar.copy(pt_sb[:, 8:16, :], pt_ps[:, 8:16, :])
                    # PV
                    o_ps = ps_o.tile([128, DV], f32, tag="o")
                    for t in range(NKT):
                        nc.tensor.matmul(o_ps, pt_sb[:, t, :], vhall[:, t, h * 64:(h + 1) * 64],
                                         start=(t == 0), stop=(t == NKT - 1))
                    nc.vector.tensor_scalar(out_sb[:, h * 64:(h + 1) * 64], o_ps, recip, None, Alu.mult)
                nc.sync.dma_start(out=out_spill[qt], in_=out_sb)

        # ================= phase 3: out-proj + layernorm =================
        with (
            tc.tile_pool(name="ph3w", bufs=1) as ph3w,
            tc.tile_pool(name="ph3", bufs=2) as ph3,
            tc.tile_pool(name="sc3", bufs=4) as sc3,
            tc.tile_pool(name="ps3", bufs=2, space="PSUM") as ps3,
            tc.tile_pool(name="ps3t", bufs=2, space="PSUM") as ps3t,
        ):
            woT_sb = ph3w.tile([128, 8, D], f32r, tag="woT")
            nc.sync.dma_start(out=woT_sb, in_=woT_e.ap().bitcast(f32r).rearrange("(m p) d -> p m d", p=128))
            for qt in range(NQT):
                qs = qt * 128
                out_l = ph3.tile([128, H * DV], f32, tag="outl")
                nc.sync.dma_start(out=out_l, in_=out_spill[qt])
                outT = ph3.tile([128, 8, 128], f32r, tag="outT")
                for m in range(8):
                    oT_ps = ps3t.tile([128, 128], f32, tag="oT")
                    nc.tensor.transpose(oT_ps, out_l[:, m * 128:(m + 1) * 128], id_f32)
                    nc.vector.tensor_copy(outT[:, m, :], oT_ps)
                qres_t = ph3.tile([128, D], f32, tag="qres")
                nc.sync.dma_start(out=qres_t, in_=qres_e[qs:qs + 128, :])
                x_sb = ph3.tile([128, D], f32, tag="x")
                for n in range(2):
                    ns = slice(n * 512, (n + 1) * 512)
                    y_ps = ps3.tile([128, 512], f32, tag="yps")
                    for m in range(8):
                        nc.tensor.matmul(y_ps, outT[:, m, :], woT_sb[:, m, ns],
                                         start=(m == 0), stop=False)
                    nc.tensor.matmul(y_ps, ones_sb, bo_sb[:, ns], start=False, stop=True)
                    nc.vector.tensor_tensor(out=x_sb[:, ns], in0=y_ps, in1=qres_t[:, ns], op=Alu.add)
                # layernorm over D=1024 (two 512 subgroups)
                x_g = x_sb.rearrange("p (g d) -> p g d", g=2)
                stats = sc3.tile([128, 2, 6], f32, tag="stats")
                for g in range(2):
                    nc.vector.bn_stats(out=stats[:, g, :], in_=x_g[:, g, :])
                mv = sc3.tile([128, 2], f32, tag="mv")
                nc.vector.bn_aggr(out=mv, in_=stats)
                std = sc3.tile([128, 1], f32, tag="std")
                nc.scalar.activation(out=std, in_=mv[:, 1:2], func=AF.Sqrt, bias=eps_sb, scale=1.0)
                rstd = sc3.tile([128, 1], f32, tag="rstd")
                nc.vector.reciprocal(rstd, std)
                xn = ph3.tile([128, D], f32, tag="xn")
                nc.vector.tensor_scalar(xn, x_sb, mv[:, 0:1], rstd, Alu.subtract, Alu.mult)
                y1 = ph3.tile([128, D], f32, tag="y1")
                nc.vector.tensor_tensor(out=y1, in0=xn, in1=lng_bc, op=Alu.mult)
                y2 = ph3.tile([128, D], f32, tag="y2")
                nc.vector.tensor_tensor(out=y2, in0=y1, in1=lnb_bc, op=Alu.add)
                nc.sync.dma_start(out=y_e[qs:qs + 128, :], in_=y2)

        dram.release()
        res.release()
        cst.release()

    nc.compile()
    return nc


def _get_nc():
    if "nc" not in _CACHE:
        _CACHE["nc"] = _build()
    return _CACHE["nc"]


def kernel(q, k, v, mask, edge, wq, bq, wk, bk, wv, bv, wo, bo, ln_g, ln_b):
    from concourse.bass_utils import run_bass_kernel_spmd

    nc = _get_nc()

    q = np.asarray(q, dtype=np.float32)
    k = np.asarray(k, dtype=np.float32)
    v = np.asarray(v, dtype=np.float32)
    mask_u8 = np.asarray(mask).astype(np.uint8)
    edge_i = np.ascontiguousarray(np.asarray(edge, dtype=np.int32))
    shared = {
        "wqT": np.ascontiguousarray(np.asarray(wq, np.float32).T),
        "wkT": np.ascontiguousarray(np.asarray(wk, np.float32).T),
        "wvT": np.ascontiguousarray(np.asarray(wv, np.float32).T),
        "woT": np.ascontiguousarray(np.asarray(wo, np.float32).T),
        "bq": np.asarray(bq, np.float32), "bk": np.asarray(bk, np.float32),
        "bv": np.asarray(bv, np.float32), "bo": np.asarray(bo, np.float32),
        "ln_g": np.asarray(ln_g, np.float32), "ln_b": np.asarray(ln_b, np.float32),
    }
    in_maps = []
    for core in range(8):
        b, q0 = core // 2, (core % 2) * QL
        m = dict(shared)
        m["qT"] = np.ascontiguousarray(q[b, q0:q0 + QL, :].T)
        m["qres"] = np.ascontiguousarray(q[b, q0:q0 + QL, :])
        m["kT"] = np.ascontiguousarray(k[b].T)
        m["vT"] = np.ascontiguousarray(v[b].T)
        m["mask"] = np.ascontiguousarray(mask_u8[b, q0:q0 + QL, :])
        m["edge"] = np.ascontiguousarray(edge_i[b, q0:q0 + QL, :])
        in_maps.append(m)

    results = run_bass_kernel_spmd(nc, in_maps, list(range(8))).results

    y = np.empty((B, L, D), np.float32)
    attn = np.empty((H * B, L, L), np.float32)
    for core in range(8):
        b, q0 = core // 2, (core % 2) * QL
        y[b, q0:q0 + QL, :] = results[core]["y"]
        a = results[core]["attn"]          # [H, QL, L]
        for h in range(H):
            attn[h * B + b, q0:q0 + QL, :] = a[h]
    return y, attn
